# revision 1
# baseline (speedup 1.0000x reference)
"""Trainium2 Bass kernel for a diagonal selective SSM layer.

Reference computation (per batch element b):
    alpha = sigmoid(x @ Wg.T + bg)        # (L, S)
    u     = x @ WB.T + bB                 # (L, S)
    h_t   = alpha_t * h_{t-1} + u_t       # scan over L, h in R^S
    y     = h @ WC.T + bC                 # (L, D)

Sharding: data-parallel over batch. B == 8 == n_cores, so each NeuronCore
processes exactly one batch element; the small projection weights are
replicated to every core. No collectives needed.

Per-core dataflow (GEMM operands in float32r = single-pass full-rate fp32
matmul mode, ~2^-13 operand rounding; everything else fp32):
  - x is fed pre-transposed as xT (D, L): contraction dim D on partitions.
  - G/U GEMMs: (S on partitions, L free) = WgT.T @ xT accumulated over 8
    k-tiles in PSUM, evicted via ScalarE activation (Sigmoid / Identity)
    with the per-partition bias.
  - The recurrence is the hardware linear-recurrence instruction
    nc.vector.tensor_tensor_scan (state = a*state + u along the free/time
    axis, fp32 internal state), chunk-chained via `initial`.
  - Output GEMM: y (L on partitions, D free) = h_tile.T @ WCT, bias bC added
    during PSUM eviction from a partition-broadcast bias tile; y DMAs out in
    natural (L, D) layout.
"""

import numpy as np

B, L, D, S = 8, 2048, 1024, 256
P = 128
NCORES = 8
KD = D // P      # 8 k-tiles over the D contraction
MS = S // P      # 2 partition groups over S
NQ = 4           # L chunks for pipelining
QL = L // NQ     # 512
LT = L // P      # 16 l-tiles for the output GEMM

# experiment knobs
GU_ORDER = "wm_outer"   # "wm_outer" | "k_outer"
WARMUP_MMS = 0          # dummy matmuls to warm the PE HAM before real work

_NC_CACHE = {}


def _build_nc():
    import concourse.mybir as mybir
    import concourse.tile as tile
    from concourse import bacc

    f32 = mybir.dt.float32
    f32r = mybir.dt.float32r
    AF = mybir.ActivationFunctionType
    OP = mybir.AluOpType

    nc = bacc.Bacc("TRN2", target_bir_lowering=False, debug=False)

    xT = nc.dram_tensor("xT", [D, L], f32, kind="ExternalInput")
    wg = nc.dram_tensor("wgT", [D, S], f32, kind="ExternalInput")
    wb = nc.dram_tensor("wbT", [D, S], f32, kind="ExternalInput")
    wc = nc.dram_tensor("wcT", [S, D], f32, kind="ExternalInput")
    bgd = nc.dram_tensor("bg2", [S, 1], f32, kind="ExternalInput")
    bbd = nc.dram_tensor("bB2", [S, 1], f32, kind="ExternalInput")
    bcd = nc.dram_tensor("bCb", [P, D], f32, kind="ExternalInput")
    y = nc.dram_tensor("y", [L, D], f32, kind="ExternalOutput")

    with tile.TileContext(nc) as tc:
        with (
            tc.tile_pool(name="persist", bufs=1) as pp,
            tc.tile_pool(name="psum", bufs=8, space="PSUM") as psp,
            tc.tile_pool(name="ystage", bufs=4) as yp,
        ):
            wgta = pp.tile([P, KD * S], f32r, name="wgta", tag="wgta")
            wbta = pp.tile([P, KD * S], f32r, name="wbta", tag="wbta")
            wcta = pp.tile([P, MS * D], f32r, name="wcta", tag="wcta")
            bgt = [pp.tile([P, 1], f32, name=f"bg{m}", tag=f"bg{m}") for m in range(MS)]
            bbt = [pp.tile([P, 1], f32, name=f"bb{m}", tag=f"bb{m}") for m in range(MS)]
            bct = pp.tile([P, D], f32, name="bc", tag="bc")
            xsa = pp.tile([P, KD * L], f32r, name="xsa", tag="xsa")
            xsa3 = xsa[:].rearrange("p (k l) -> p k l", k=KD)
            xT3 = xT[:, :].bitcast(f32r).rearrange("(k p) l -> p k l", p=P)

            def dma_x_chunk(q, k=None):
                # one issue moves the q-th L-chunk of ALL k-tiles (2 MB)
                nc.sync.dma_start(
                    xsa3[:, :, q * QL:(q + 1) * QL],
                    xT3[:, :, q * QL:(q + 1) * QL],
                )
            xs = [xsa[:, k * L:(k + 1) * L] for k in range(KD)]

            # issue order matters: the single HWDGE issue queue serializes at
            # ~0.6us per dma_start, so interleave the first L-chunk of x with
            # the weights the first GEMM group needs.
            dma_x_chunk(0)
            for k in range(KD):
                nc.sync.dma_start(wgta[:, k * S:(k + 1) * S], wg[k * P:(k + 1) * P, :].bitcast(f32r))
            dma_x_chunk(1)
            for k in range(KD):
                nc.sync.dma_start(wbta[:, k * S:(k + 1) * S], wb[k * P:(k + 1) * P, :].bitcast(f32r))
            dma_x_chunk(2)
            for m in range(MS):
                nc.sync.dma_start(bgt[m][:], bgd[m * P:(m + 1) * P, :])
                nc.sync.dma_start(bbt[m][:], bbd[m * P:(m + 1) * P, :])
            for q in range(3, NQ):
                dma_x_chunk(q)
            for m in range(MS):
                nc.sync.dma_start(wcta[:, m * D:(m + 1) * D], wc[m * P:(m + 1) * P, :].bitcast(f32r))
            nc.sync.dma_start(bct[:], bcd[:, :])
            wgt = [wgta[:, k * S:(k + 1) * S] for k in range(KD)]
            wbt = [wbta[:, k * S:(k + 1) * S] for k in range(KD)]
            wct = [wcta[:, m * D:(m + 1) * D] for m in range(MS)]

            alpha = [pp.tile([P, L], f32, name=f"al{m}", tag=f"al{m}") for m in range(MS)]
            uu = [pp.tile([P, L], f32, name=f"uu{m}", tag=f"uu{m}") for m in range(MS)]
            hh = [pp.tile([P, L], f32r, name=f"hh{m}", tag=f"hh{m}") for m in range(MS)]

            if WARMUP_MMS:
                # PE HAM warm-up on already-resident weight tiles while the
                # x DMA streams in; result is discarded.
                wps = psp.tile([P, QL], f32, name="wps", tag="ps")
                for i in range(WARMUP_MMS):
                    nc.tensor.matmul(
                        wps[:, 0:S], wgt[0][:, 0:P], wgt[0][:, 0:S],
                        start=(i == 0), stop=(i == WARMUP_MMS - 1),
                    )

            groups = [
                (wgt, bgt, alpha, AF.Sigmoid, 0),
                (wgt, bgt, alpha, AF.Sigmoid, 1),
                (wbt, bbt, uu, AF.Identity, 0),
                (wbt, bbt, uu, AF.Identity, 1),
            ]
            for q in range(NQ):
                qs = slice(q * QL, (q + 1) * QL)
                if GU_ORDER == "wm_outer":
                    for wt, bt, dst, fn, m in groups:
                        ps = psp.tile([P, QL], f32, name="ps", tag="ps")
                        for k in range(KD):
                            nc.tensor.matmul(
                                ps[:],
                                wt[k][:, m * P:(m + 1) * P],
                                xs[k][:, qs],
                                start=(k == 0),
                                stop=(k == KD - 1),
                            )
                        nc.scalar.activation(
                            dst[m][:, qs], ps[:], fn, bias=bt[m][:, 0:1], scale=1.0
                        )
                else:  # k_outer: 4 live accumulators, PE consumes x as it lands
                    pss = [psp.tile([P, QL], f32, name="ps", tag="ps") for _ in groups]
                    for k in range(KD):
                        for gi, (wt, bt, dst, fn, m) in enumerate(groups):
                            nc.tensor.matmul(
                                pss[gi][:],
                                wt[k][:, m * P:(m + 1) * P],
                                xs[k][:, qs],
                                start=(k == 0),
                                stop=(k == KD - 1),
                            )
                    for gi, (wt, bt, dst, fn, m) in enumerate(groups):
                        nc.scalar.activation(
                            dst[m][:, qs], pss[gi][:], fn, bias=bt[m][:, 0:1], scale=1.0
                        )
                # chunk-chained hardware scan: state = alpha*state + u
                for m in range(MS):
                    init = 0.0 if q == 0 else hh[m][:, q * QL - 1:q * QL]
                    nc.vector.tensor_tensor_scan(
                        hh[m][:, qs], alpha[m][:, qs], uu[m][:, qs],
                        init, OP.mult, OP.add,
                    )
                # output GEMM for the l-tiles of this chunk
                for l in range(q * (LT // NQ), (q + 1) * (LT // NQ)):
                    ls = slice(l * P, (l + 1) * P)
                    ysb = yp.tile([P, D], f32, name="ysb", tag="ysb")
                    for nn in range(2):
                        ns = slice(nn * 512, (nn + 1) * 512)
                        ps = psp.tile([P, 512], f32, name="psy", tag="ps")
                        for m in range(MS):
                            nc.tensor.matmul(
                                ps[:],
                                hh[m][:, ls],
                                wct[m][:, ns],
                                start=(m == 0),
                                stop=(m == MS - 1),
                            )
                        nc.vector.tensor_tensor(ysb[:, ns], ps[:], bct[:, ns], OP.add)
                    nc.sync.dma_start(y[ls, :], ysb[:])

    nc.finalize()
    return nc


def _get_nc():
    if "nc" not in _NC_CACHE:
        _NC_CACHE["nc"] = _build_nc()
    return _NC_CACHE["nc"]


def _make_in_maps(x, Wg, bg, WB, bB, WC, bC):
    x = np.ascontiguousarray(np.asarray(x, dtype=np.float32))
    wgT = np.ascontiguousarray(np.asarray(Wg, dtype=np.float32).T)
    wbT = np.ascontiguousarray(np.asarray(WB, dtype=np.float32).T)
    wcT = np.ascontiguousarray(np.asarray(WC, dtype=np.float32).T)
    bg2 = np.ascontiguousarray(np.asarray(bg, dtype=np.float32).reshape(S, 1))
    bb2 = np.ascontiguousarray(np.asarray(bB, dtype=np.float32).reshape(S, 1))
    bcb = np.ascontiguousarray(
        np.broadcast_to(np.asarray(bC, dtype=np.float32).reshape(1, D), (P, D))
    )
    in_maps = []
    for b in range(NCORES):
        in_maps.append({
            "xT": np.ascontiguousarray(x[b].T),
            "wgT": wgT,
            "wbT": wbT,
            "wcT": wcT,
            "bg2": bg2,
            "bB2": bb2,
            "bCb": bcb,
        })
    return in_maps


def _run(in_maps, **kwargs):
    from concourse.bass_utils import run_bass_kernel_spmd

    nc = _get_nc()
    return run_bass_kernel_spmd(nc, in_maps, list(range(NCORES)), **kwargs)


def kernel(x, Wg, bg, WB, bB, WC, bC):
    res = _run(_make_in_maps(x, Wg, bg, WB, bB, WC, bC))
    out = np.stack([res.results[b]["y"] for b in range(NCORES)])
    return np.ascontiguousarray(out.astype(np.float32, copy=False))



# revision 2
# speedup vs baseline: 1.5291x; 1.5291x over previous
"""Trainium2 Bass kernel for a diagonal selective SSM layer.

Reference computation (per batch element b):
    alpha = sigmoid(x @ Wg.T + bg)        # (L, S)
    u     = x @ WB.T + bB                 # (L, S)
    h_t   = alpha_t * h_{t-1} + u_t       # scan over L, h in R^S
    y     = h @ WC.T + bC                 # (L, D)

Sharding: data-parallel over batch. B == 8 == n_cores, so each NeuronCore
processes exactly one batch element; the small projection weights are
replicated to every core. No collectives needed.

Per-core dataflow:
  - G/U GEMMs in bf16 (x, Wg, WB shipped as bf16 from the host: halves HBM
    traffic, enables FWL weight loads; the PE streams 1 col/cycle for bf16
    and f32r alike so matmul time is unchanged). PSUM fp32 accumulation.
  - alpha eviction: ScalarE Sigmoid activation with per-partition bias.
    u eviction: VectorE tensor_scalar_add with per-partition bias.
  - Recurrence: hardware linear-recurrence nc.vector.tensor_tensor_scan
    (state = a*state + u, fp32 internal state), chunk-chained via `initial`.
  - Output GEMM transposed (yT layout, D on partitions): the scan output
    hh (S on partitions, L free) is the moving operand, WC tiles (S parts,
    D free) the stationary one.  bias bC is then per-partition and fuses
    into the PSUM eviction (split ScalarE activation / VectorE
    tensor_scalar_add), which also casts to bf16.  yT DMAs out as (D, L)
    bf16, one issue per L-chunk; the host transposes/upcasts.
  - HAM warm-up: a burst of dummy matmuls on memset tiles keeps the PE
    busy during the initial DMA wait so real matmuls run at 2.4 GHz.
  - Y GEMMs are skewed one chunk behind the G/U GEMMs so the PE never
    waits on the scan.
"""

import numpy as np

B, L, D, S = 8, 2048, 1024, 256
P = 128
NCORES = 8
KD = D // P      # 8 k-tiles over the D contraction
MS = S // P      # 2 partition groups over S
DT = D // P      # 8 output D-tiles
NQ = 4           # L chunks for pipelining
QL = L // NQ     # 512

WARMUP_MMS = 8   # dummy matmuls to warm the PE HAM before real work

_NC_CACHE = {}


def _build_nc():
    import concourse.mybir as mybir
    import concourse.tile as tile
    from concourse import bacc

    f32 = mybir.dt.float32
    f32r = mybir.dt.float32r
    bf16 = mybir.dt.bfloat16
    AF = mybir.ActivationFunctionType
    OP = mybir.AluOpType

    nc = bacc.Bacc("TRN2", target_bir_lowering=False, debug=False)

    xT = nc.dram_tensor("xT", [D, L], bf16, kind="ExternalInput")
    wg = nc.dram_tensor("wgT", [D, S], bf16, kind="ExternalInput")
    wb = nc.dram_tensor("wbT", [D, S], bf16, kind="ExternalInput")
    wc = nc.dram_tensor("wcT", [S, D], f32, kind="ExternalInput")
    bias = nc.dram_tensor("biasP", [P, 4 + DT], f32, kind="ExternalInput")
    y = nc.dram_tensor("yT", [D, L], bf16, kind="ExternalOutput")

    with tile.TileContext(nc) as tc:
        with (
            tc.tile_pool(name="persist", bufs=1) as pp,
            tc.tile_pool(name="psum", bufs=8, space="PSUM") as psp,
        ):
            wgta = pp.tile([P, KD * S], bf16, name="wgta", tag="wgta")
            wbta = pp.tile([P, KD * S], bf16, name="wbta", tag="wbta")
            wcta = pp.tile([P, MS * D], f32r, name="wcta", tag="wcta")
            biast = pp.tile([P, 4 + DT], f32, name="biast", tag="biast")
            xsa = pp.tile([P, KD * L], bf16, name="xsa", tag="xsa")
            xsa3 = xsa[:].rearrange("p (k l) -> p k l", k=KD)
            xT3 = xT[:, :].rearrange("(k p) l -> p k l", p=P)
            ysta = pp.tile([P, DT * L], bf16, name="ysta", tag="ysta")
            ysta3 = ysta[:].rearrange("p (t l) -> p t l", t=DT)
            yT3 = y[:, :].rearrange("(t p) l -> p t l", p=P)

            # PE warm-up fodder (no DMA dependencies)
            wul = pp.tile([P, P], bf16, name="wul", tag="wul")
            wur = pp.tile([P, QL], bf16, name="wur", tag="wur")

            def dma_x_chunk(q):
                # one issue moves the q-th L-chunk of ALL k-tiles (1 MB bf16)
                nc.sync.dma_start(
                    xsa3[:, :, q * QL:(q + 1) * QL],
                    xT3[:, :, q * QL:(q + 1) * QL],
                )

            # Few, large DMA issues: the HWDGE issue queue serializes at
            # ~0.9us per dma_start.
            dma_x_chunk(0)
            nc.sync.dma_start(
                wgta[:].rearrange("p (k s) -> p k s", k=KD),
                wg[:, :].rearrange("(k p) s -> p k s", p=P),
            )
            nc.sync.dma_start(
                wbta[:].rearrange("p (k s) -> p k s", k=KD),
                wb[:, :].rearrange("(k p) s -> p k s", p=P),
            )
            nc.sync.dma_start(biast[:], bias[:, :])
            dma_x_chunk(1)
            nc.sync.dma_start(
                wcta[:].rearrange("p (m d) -> p m d", m=MS),
                wc[:, :].bitcast(f32r).rearrange("(m p) d -> p m d", p=P),
            )
            dma_x_chunk(2)
            dma_x_chunk(3)

            alpha = [pp.tile([P, L], f32, name=f"al{m}", tag=f"al{m}") for m in range(MS)]
            uu = [pp.tile([P, L], f32, name=f"uu{m}", tag=f"uu{m}") for m in range(MS)]
            hh = [pp.tile([P, L], f32r, name=f"hh{m}", tag=f"hh{m}") for m in range(MS)]

            if WARMUP_MMS:
                nc.vector.memset(wul[:], 0.0)
                nc.vector.memset(wur[:], 0.0)
                wps = psp.tile([P, QL], f32, name="wps", tag="ps")
                for i in range(WARMUP_MMS):
                    nc.tensor.matmul(
                        wps[:], wul[:], wur[:],
                        start=(i == 0), stop=(i == WARMUP_MMS - 1),
                    )

            # groups: (weight tile base, dst, m, is_sigmoid)
            groups = [
                ("g", alpha, 0), ("g", alpha, 1),
                ("b", uu, 0), ("b", uu, 1),
            ]

            def emit_gu(q):
                qs = slice(q * QL, (q + 1) * QL)
                for wt, dst, m in groups:
                    wta = wgta if wt == "g" else wbta
                    ps = psp.tile([P, QL], f32, name="ps", tag="ps")
                    for k in range(KD):
                        nc.tensor.matmul(
                            ps[:],
                            wta[:, k * S + m * P:k * S + (m + 1) * P],
                            xsa[:, k * L + q * QL:k * L + (q + 1) * QL],
                            start=(k == 0),
                            stop=(k == KD - 1),
                        )
                    if wt == "g":
                        nc.scalar.activation(
                            dst[m][:, qs], ps[:], AF.Sigmoid,
                            bias=biast[:, m:m + 1], scale=1.0,
                        )
                    else:
                        nc.vector.tensor_scalar_add(
                            dst[m][:, qs], ps[:], biast[:, 2 + m:3 + m],
                        )
                # chunk-chained hardware scan: state = alpha*state + u
                for m in range(MS):
                    init = 0.0 if q == 0 else hh[m][:, q * QL - 1:q * QL]
                    nc.vector.tensor_tensor_scan(
                        hh[m][:, qs], alpha[m][:, qs], uu[m][:, qs],
                        init, OP.mult, OP.add,
                    )

            def emit_y(q):
                qs = slice(q * QL, (q + 1) * QL)
                for t in range(DT):
                    ps = psp.tile([P, QL], f32, name="psy", tag="ps")
                    for m in range(MS):
                        nc.tensor.matmul(
                            ps[:],
                            wcta[:, m * D + t * P:m * D + (t + 1) * P],
                            hh[m][:, qs],
                            start=(m == 0),
                            stop=(m == MS - 1),
                        )
                    dst = ysta[:, t * L + q * QL:t * L + (q + 1) * QL]
                    bc = biast[:, 4 + t:5 + t]
                    if t % 2 == 0:
                        nc.scalar.activation(dst, ps[:], AF.Identity, bias=bc, scale=1.0)
                    else:
                        nc.vector.tensor_scalar_add(dst, ps[:], bc)
                nc.sync.dma_start(yT3[:, :, qs], ysta3[:, :, qs])

            # software pipeline: Y GEMMs run one chunk behind G/U GEMMs so
            # the PE never waits on the scan.
            emit_gu(0)
            for q in range(1, NQ):
                emit_gu(q)
                emit_y(q - 1)
            emit_y(NQ - 1)

    nc.finalize()
    return nc


def _get_nc():
    if "nc" not in _NC_CACHE:
        _NC_CACHE["nc"] = _build_nc()
    return _NC_CACHE["nc"]


def _make_in_maps(x, Wg, bg, WB, bB, WC, bC):
    import ml_dtypes

    bf16 = ml_dtypes.bfloat16
    x = np.asarray(x, dtype=np.float32)
    wgT = np.ascontiguousarray(np.asarray(Wg, dtype=np.float32).T.astype(bf16))
    wbT = np.ascontiguousarray(np.asarray(WB, dtype=np.float32).T.astype(bf16))
    wcT = np.ascontiguousarray(np.asarray(WC, dtype=np.float32).T)
    bias = np.zeros((P, 4 + DT), dtype=np.float32)
    bias[:, 0] = np.asarray(bg, dtype=np.float32)[0:P]
    bias[:, 1] = np.asarray(bg, dtype=np.float32)[P:2 * P]
    bias[:, 2] = np.asarray(bB, dtype=np.float32)[0:P]
    bias[:, 3] = np.asarray(bB, dtype=np.float32)[P:2 * P]
    bias[:, 4:] = np.asarray(bC, dtype=np.float32).reshape(DT, P).T
    in_maps = []
    for b in range(NCORES):
        in_maps.append({
            "xT": np.ascontiguousarray(x[b].T.astype(bf16)),
            "wgT": wgT,
            "wbT": wbT,
            "wcT": wcT,
            "biasP": bias,
        })
    return in_maps


def _run(in_maps, **kwargs):
    from concourse.bass_utils import run_bass_kernel_spmd

    nc = _get_nc()
    return run_bass_kernel_spmd(nc, in_maps, list(range(NCORES)), **kwargs)


def kernel(x, Wg, bg, WB, bB, WC, bC):
    res = _run(_make_in_maps(x, Wg, bg, WB, bB, WC, bC))
    out = np.stack([
        np.asarray(res.results[b]["yT"]).astype(np.float32).T
        for b in range(NCORES)
    ])
    return np.ascontiguousarray(out)


# revision 7
# speedup vs baseline: 1.5841x; 1.0359x over previous
"""Trainium2 Bass kernel for a diagonal selective SSM layer.

Reference computation (per batch element b):
    alpha = sigmoid(x @ Wg.T + bg)        # (L, S)
    u     = x @ WB.T + bB                 # (L, S)
    h_t   = alpha_t * h_{t-1} + u_t       # scan over L, h in R^S
    y     = h @ WC.T + bC                 # (L, D)

Sharding: data-parallel over batch. B == 8 == n_cores, so each NeuronCore
processes exactly one batch element; the small projection weights are
replicated to every core. No collectives needed.

Per-core dataflow:
  - G/U GEMMs in bf16 (x, Wg, WB shipped as bf16 from the host: halves HBM
    traffic, enables FWL weight loads; the PE streams 1 col/cycle for bf16
    and f32r alike so matmul time is unchanged). PSUM fp32 accumulation.
  - alpha eviction: ScalarE Sigmoid activation with per-partition bias.
    u eviction: VectorE tensor_scalar_add with per-partition bias.
  - Recurrence: hardware linear-recurrence nc.vector.tensor_tensor_scan
    (state = a*state + u, fp32 internal state), chunk-chained via `initial`.
  - Output GEMM transposed (yT layout, D on partitions): the scan output
    hh (S on partitions, L free) is the moving operand, WC tiles (S parts,
    D free) the stationary one.  bias bC is then per-partition and fuses
    into the PSUM eviction (split ScalarE activation / VectorE
    tensor_scalar_add), which also casts to bf16.  yT DMAs out as (D, L)
    bf16, one issue per L-chunk; the host transposes/upcasts.
  - HAM warm-up: a burst of dummy matmuls on memset tiles keeps the PE
    busy during the initial DMA wait so real matmuls run at 2.4 GHz.
  - Y GEMMs are skewed one chunk behind the G/U GEMMs so the PE never
    waits on the scan.
"""

import numpy as np

B, L, D, S = 8, 2048, 1024, 256
P = 128
NCORES = 8
KD = D // P      # 8 k-tiles over the D contraction
MS = S // P      # 2 partition groups over S
DT = D // P      # 8 output D-tiles

# L chunks: small first chunk so the first GEMM starts as soon as possible
# after the x/Wg DMAs land; small last chunk to shorten the scan->Y->DMA tail.
CHUNKS = [256, 512, 512, 512, 256]
OFFS = [sum(CHUNKS[:i]) for i in range(len(CHUNKS) + 1)]
assert OFFS[-1] == L

WARMUP_MMS = 14  # dummy matmuls to warm the PE HAM before real work

_NC_CACHE = {}


def _build_nc():
    import concourse.mybir as mybir
    import concourse.tile as tile
    from concourse import bacc

    f32 = mybir.dt.float32
    f32r = mybir.dt.float32r
    bf16 = mybir.dt.bfloat16
    AF = mybir.ActivationFunctionType
    OP = mybir.AluOpType

    nc = bacc.Bacc("TRN2", target_bir_lowering=False, debug=False)

    xT = nc.dram_tensor("xT", [D, L], bf16, kind="ExternalInput")
    wg = nc.dram_tensor("wgT", [D, S], bf16, kind="ExternalInput")
    wb = nc.dram_tensor("wbT", [D, S], bf16, kind="ExternalInput")
    wc = nc.dram_tensor("wcT", [S, D], f32, kind="ExternalInput")
    bias = nc.dram_tensor("biasP", [P, 4 + DT], f32, kind="ExternalInput")
    y = nc.dram_tensor("yT", [D, L], bf16, kind="ExternalOutput")

    with tile.TileContext(nc) as tc:
        with (
            tc.tile_pool(name="persist", bufs=1) as pp,
            tc.tile_pool(name="psum", bufs=8, space="PSUM") as psp,
        ):
            wgta = pp.tile([P, KD * S], bf16, name="wgta", tag="wgta")
            wbta = pp.tile([P, KD * S], bf16, name="wbta", tag="wbta")
            wcta = pp.tile([P, MS * D], f32r, name="wcta", tag="wcta")
            biast = pp.tile([P, 4 + DT], f32, name="biast", tag="biast")
            xsa = pp.tile([P, KD * L], bf16, name="xsa", tag="xsa")
            xsa3 = xsa[:].rearrange("p (k l) -> p k l", k=KD)
            xT3 = xT[:, :].rearrange("(k p) l -> p k l", p=P)
            ysta = pp.tile([P, DT * L], bf16, name="ysta", tag="ysta")
            ysta3 = ysta[:].rearrange("p (t l) -> p t l", t=DT)
            yT3 = y[:, :].rearrange("(t p) l -> p t l", p=P)

            # PE warm-up fodder (no DMA dependencies)
            wul = pp.tile([P, P], bf16, name="wul", tag="wul")
            wur = pp.tile([P, 512], bf16, name="wur", tag="wur")

            def dma_x_chunk(q):
                # one issue moves the q-th L-chunk of ALL k-tiles
                nc.sync.dma_start(
                    xsa3[:, :, OFFS[q]:OFFS[q + 1]],
                    xT3[:, :, OFFS[q]:OFFS[q + 1]],
                )

            # Few, large DMA issues (the HWDGE issue queue serializes at
            # ~1us per dma_start), ordered by when the consumer needs them.
            nc.sync.dma_start(
                wgta[:].rearrange("p (k s) -> p k s", k=KD),
                wg[:, :].rearrange("(k p) s -> p k s", p=P),
            )
            dma_x_chunk(0)
            nc.sync.dma_start(
                wbta[:].rearrange("p (k s) -> p k s", k=KD),
                wb[:, :].rearrange("(k p) s -> p k s", p=P),
            )
            dma_x_chunk(1)
            nc.sync.dma_start(biast[:], bias[:, :])
            dma_x_chunk(2)
            nc.sync.dma_start(
                wcta[:].rearrange("p (m d) -> p m d", m=MS),
                wc[:, :].bitcast(f32r).rearrange("(m p) d -> p m d", p=P),
            )
            for q in range(3, len(CHUNKS)):
                dma_x_chunk(q)

            alpha = [pp.tile([P, L], f32, name=f"al{m}", tag=f"al{m}") for m in range(MS)]
            uu = [pp.tile([P, L], f32, name=f"uu{m}", tag=f"uu{m}") for m in range(MS)]
            hh = [pp.tile([P, L], f32r, name=f"hh{m}", tag=f"hh{m}") for m in range(MS)]

            if WARMUP_MMS:
                nc.vector.memset(wul[:], 0.0)
                nc.vector.memset(wur[:], 0.0)
                wps = psp.tile([P, 512], f32, name="wps", tag="ps")
                for i in range(WARMUP_MMS):
                    nc.tensor.matmul(
                        wps[:], wul[:], wur[:],
                        start=(i == 0), stop=(i == WARMUP_MMS - 1),
                    )

            # groups: (weight tile base, dst, m, is_sigmoid)
            groups = [
                ("g", alpha, 0), ("g", alpha, 1),
                ("b", uu, 0), ("b", uu, 1),
            ]

            def emit_gu(q):
                o0, o1 = OFFS[q], OFFS[q + 1]
                cl = o1 - o0
                qs = slice(o0, o1)
                for wt, dst, m in groups:
                    wta = wgta if wt == "g" else wbta
                    ps = psp.tile([P, 512], f32, name="ps", tag="ps")
                    for k in range(KD):
                        nc.tensor.matmul(
                            ps[:, :cl],
                            wta[:, k * S + m * P:k * S + (m + 1) * P],
                            xsa[:, k * L + o0:k * L + o1],
                            start=(k == 0),
                            stop=(k == KD - 1),
                        )
                    if wt == "g":
                        nc.scalar.activation(
                            dst[m][:, qs], ps[:, :cl], AF.Sigmoid,
                            bias=biast[:, m:m + 1], scale=1.0,
                        )
                    else:
                        nc.vector.tensor_scalar_add(
                            dst[m][:, qs], ps[:, :cl], biast[:, 2 + m:3 + m],
                        )
                # chunk-chained hardware scan: state = alpha*state + u
                for m in range(MS):
                    init = 0.0 if q == 0 else hh[m][:, o0 - 1:o0]
                    nc.vector.tensor_tensor_scan(
                        hh[m][:, qs], alpha[m][:, qs], uu[m][:, qs],
                        init, OP.mult, OP.add,
                    )

            def emit_y(q):
                o0, o1 = OFFS[q], OFFS[q + 1]
                cl = o1 - o0
                qs = slice(o0, o1)
                last = q == len(CHUNKS) - 1
                for t in range(DT):
                    ps = psp.tile([P, 512], f32, name="psy", tag="ps")
                    for m in range(MS):
                        nc.tensor.matmul(
                            ps[:, :cl],
                            wcta[:, m * D + t * P:m * D + (t + 1) * P],
                            hh[m][:, qs],
                            start=(m == 0),
                            stop=(m == MS - 1),
                        )
                    dst = ysta[:, t * L + o0:t * L + o1]
                    bc = biast[:, 4 + t:5 + t]
                    if t % 2 == 0:
                        nc.scalar.activation(dst, ps[:, :cl], AF.Identity, bias=bc, scale=1.0)
                    else:
                        nc.vector.tensor_scalar_add(dst, ps[:, :cl], bc)
                    if last and t == DT // 2 - 1:
                        # get the first half of the final writeback moving early
                        nc.sync.dma_start(
                            yT3[:, 0:DT // 2, qs], ysta3[:, 0:DT // 2, qs]
                        )
                if last:
                    nc.sync.dma_start(
                        yT3[:, DT // 2:DT, qs], ysta3[:, DT // 2:DT, qs]
                    )
                else:
                    nc.sync.dma_start(yT3[:, :, qs], ysta3[:, :, qs])

            # software pipeline: Y GEMMs run one chunk behind G/U GEMMs so
            # the PE never waits on the scan.
            emit_gu(0)
            for q in range(1, len(CHUNKS)):
                emit_gu(q)
                emit_y(q - 1)
            emit_y(len(CHUNKS) - 1)

    nc.finalize()
    return nc


def _get_nc():
    if "nc" not in _NC_CACHE:
        _NC_CACHE["nc"] = _build_nc()
    return _NC_CACHE["nc"]


def _make_in_maps(x, Wg, bg, WB, bB, WC, bC):
    import ml_dtypes

    bf16 = ml_dtypes.bfloat16
    x = np.asarray(x, dtype=np.float32)
    wgT = np.ascontiguousarray(np.asarray(Wg, dtype=np.float32).T.astype(bf16))
    wbT = np.ascontiguousarray(np.asarray(WB, dtype=np.float32).T.astype(bf16))
    wcT = np.ascontiguousarray(np.asarray(WC, dtype=np.float32).T)
    bias = np.zeros((P, 4 + DT), dtype=np.float32)
    bias[:, 0] = np.asarray(bg, dtype=np.float32)[0:P]
    bias[:, 1] = np.asarray(bg, dtype=np.float32)[P:2 * P]
    bias[:, 2] = np.asarray(bB, dtype=np.float32)[0:P]
    bias[:, 3] = np.asarray(bB, dtype=np.float32)[P:2 * P]
    bias[:, 4:] = np.asarray(bC, dtype=np.float32).reshape(DT, P).T
    in_maps = []
    for b in range(NCORES):
        in_maps.append({
            "xT": np.ascontiguousarray(x[b].T.astype(bf16)),
            "wgT": wgT,
            "wbT": wbT,
            "wcT": wcT,
            "biasP": bias,
        })
    return in_maps


def _run(in_maps, **kwargs):
    from concourse.bass_utils import run_bass_kernel_spmd

    nc = _get_nc()
    return run_bass_kernel_spmd(nc, in_maps, list(range(NCORES)), **kwargs)


def kernel(x, Wg, bg, WB, bB, WC, bC):
    res = _run(_make_in_maps(x, Wg, bg, WB, bB, WC, bC))
    out = np.stack([
        np.asarray(res.results[b]["yT"]).astype(np.float32).T
        for b in range(NCORES)
    ])
    return np.ascontiguousarray(out)
